# revision 3
# baseline (speedup 1.0000x reference)
"""BerHu (reverse Huber) loss on 8 Trainium2 NeuronCores.

Reference computation (jax, fp32):
    diff = |target - input|                  # [32, 1, 480, 640]
    c = 0.2 * max(diff)
    per_pixel = where(diff <= c, diff, (diff^2 + c^2) / (2c))
    out = sum(per_pixel) / 32

Identity used to avoid the select:
    berhu(x) = x + relu(x - c)^2 / (2c)      for x = |diff| >= 0
(check: x <= c -> x; x > c -> x + (x-c)^2/(2c) = (x^2 + c^2)/(2c))

Sharding: data-parallel over the batch dim (4 images per core). Each core
computes its per-partition |diff| (resident in SBUF), local abs-max and
local sum(|diff|); an AllReduce(max) produces the global threshold c; a
second pass over the SBUF-resident |diff| accumulates sum(relu(x-c)^2);
an AllReduce(add) combines the per-core partials. Every core emits the
identical final scalar; the host returns core 0's.
"""

import sys

import numpy as np

if "/opt/trn_rl_repo" not in sys.path:
    sys.path.insert(0, "/opt/trn_rl_repo")

N_CORES = 8
B, H, W = 32, 480, 640
P = 128                       # SBUF partitions
PER_CORE = (B // N_CORES) * H * W   # 1228800 elements per core
FREE = PER_CORE // P          # 9600 columns per partition
NT = 8                        # pipeline tiles per tensor
F = FREE // NT                # 1200 columns per tile

_PROGRAM_CACHE: dict = {}


def build_program(n_cores: int = N_CORES, free: int = FREE, nt: int = NT):
    """Emit the SPMD Bass program (identical on every core)."""
    import concourse.bass as bass
    import concourse.mybir as mybir
    import concourse.tile as tile
    from concourse import bacc, bass_isa

    f32 = mybir.dt.float32
    f = free // nt
    alu = mybir.AluOpType
    act = mybir.ActivationFunctionType
    group = [list(range(n_cores))]

    nc = bacc.Bacc(
        "TRN2", target_bir_lowering=False, debug=False, num_devices=n_cores
    )
    inp = nc.dram_tensor("input", [P, free], f32, kind="ExternalInput").ap()
    tgt = nc.dram_tensor("target", [P, free], f32, kind="ExternalInput").ap()
    out = nc.dram_tensor("output", [1, 1], f32, kind="ExternalOutput").ap()

    with tile.TileContext(nc) as tc:
        with (
            tc.tile_pool(name="io", bufs=3) as io_pool,
            tc.tile_pool(name="work", bufs=2) as work_pool,
            tc.tile_pool(name="res", bufs=1) as res_pool,
            tc.tile_pool(name="dram", bufs=1, space="DRAM") as dram,
        ):
            # |diff| stays resident so pass 2 never touches HBM.
            xabs = res_pool.tile([P, free], f32)
            amax_cols = res_pool.tile([P, nt], f32)
            asum_cols = res_pool.tile([P, nt], f32)
            rsum_cols = res_pool.tile([P, nt], f32)

            # ---- pass 1: d = target - input, per-tile abs-max, sum |d| ----
            for j in range(nt):
                sl = slice(j * f, (j + 1) * f)
                tin = io_pool.tile([P, f], f32, tag="tin")
                ttg = io_pool.tile([P, f], f32, tag="ttg")
                d = work_pool.tile([P, f], f32, tag="d")
                nc.sync.dma_start(out=tin[:], in_=inp[:, sl])
                nc.sync.dma_start(out=ttg[:], in_=tgt[:, sl])
                nc.vector.tensor_sub(d[:], ttg[:], tin[:])
                nc.vector.tensor_reduce(
                    out=amax_cols[:, j : j + 1],
                    in_=d[:],
                    axis=mybir.AxisListType.X,
                    op=alu.max,
                    apply_absolute_value=True,
                )
                nc.scalar.activation(
                    out=xabs[:, sl],
                    in_=d[:],
                    func=act.Abs,
                    accum_out=asum_cols[:, j : j + 1],
                )

            # ---- global threshold c = 0.2 * allreduce_max(|d|) ----
            amax_p = res_pool.tile([P, 1], f32)
            nc.vector.tensor_reduce(
                out=amax_p[:], in_=amax_cols[:], axis=mybir.AxisListType.X,
                op=alu.max,
            )
            cc_max_in = dram.tile([P, 1], f32)
            cc_max_out = dram.tile([P, 1], f32, addr_space="Shared")
            nc.sync.dma_start(out=cc_max_in[:], in_=amax_p[:])
            nc.gpsimd.collective_compute(
                "AllReduce",
                alu.max,
                replica_groups=group,
                ins=[cc_max_in.opt()],
                outs=[cc_max_out.opt()],
            )
            gmax = res_pool.tile([P, 1], f32)
            nc.sync.dma_start(out=gmax[:], in_=cc_max_out[:])
            # every partition gets the global max
            nc.gpsimd.partition_all_reduce(
                gmax[:], gmax[:], P, bass_isa.ReduceOp.max
            )

            c_b = res_pool.tile([P, 1], f32)
            neg_c = res_pool.tile([P, 1], f32)
            inv2c = res_pool.tile([P, 1], f32)
            nc.scalar.mul(c_b[:], gmax[:], 0.2)
            nc.scalar.mul(neg_c[:], gmax[:], -0.2)
            nc.scalar.mul(inv2c[:], gmax[:], 0.4)
            nc.vector.reciprocal(inv2c[:], inv2c[:])

            # ---- pass 2: sum relu(x - c)^2 over resident |d| ----
            for j in range(nt):
                sl = slice(j * f, (j + 1) * f)
                u = work_pool.tile([P, f], f32, tag="u")
                sq = work_pool.tile([P, f], f32, tag="sq")
                nc.vector.tensor_scalar(
                    out=u[:],
                    in0=xabs[:, sl],
                    scalar1=c_b[:],
                    scalar2=None,
                    op0=alu.max,
                )
                nc.scalar.activation(
                    out=sq[:],
                    in_=u[:],
                    func=act.Square,
                    bias=neg_c[:],
                    scale=1.0,
                    accum_out=rsum_cols[:, j : j + 1],
                )

            # ---- combine: part = sum|d| + relu_sq_sum / (2c), per partition ----
            a_p = res_pool.tile([P, 1], f32)
            r_p = res_pool.tile([P, 1], f32)
            part = res_pool.tile([P, 1], f32)
            nc.vector.tensor_reduce(
                out=a_p[:], in_=asum_cols[:], axis=mybir.AxisListType.X,
                op=alu.add,
            )
            nc.vector.tensor_reduce(
                out=r_p[:], in_=rsum_cols[:], axis=mybir.AxisListType.X,
                op=alu.add,
            )
            # part = (r_p * inv2c) + a_p
            nc.vector.scalar_tensor_tensor(
                out=part[:],
                in0=r_p[:],
                scalar=inv2c[:],
                in1=a_p[:],
                op0=alu.mult,
                op1=alu.add,
            )

            # ---- global sum across partitions and cores ----
            cc_sum_in = dram.tile([P, 1], f32)
            cc_sum_out = dram.tile([P, 1], f32, addr_space="Shared")
            nc.sync.dma_start(out=cc_sum_in[:], in_=part[:])
            nc.gpsimd.collective_compute(
                "AllReduce",
                alu.add,
                replica_groups=group,
                ins=[cc_sum_in.opt()],
                outs=[cc_sum_out.opt()],
            )
            tot = res_pool.tile([P, 1], f32)
            nc.sync.dma_start(out=tot[:], in_=cc_sum_out[:])
            nc.gpsimd.partition_all_reduce(
                tot[:], tot[:], P, bass_isa.ReduceOp.add
            )
            final = res_pool.tile([1, 1], f32)
            nc.scalar.mul(final[:], tot[0:1, :], 1.0 / B)
            nc.sync.dma_start(out=out[:], in_=final[:])

    nc.compile()
    return nc


def _get_program():
    key = (N_CORES, FREE, NT)
    if key not in _PROGRAM_CACHE:
        _PROGRAM_CACHE[key] = build_program()
    return _PROGRAM_CACHE[key]


def shard_inputs(input: np.ndarray, target: np.ndarray):
    per_b = B // N_CORES
    in_maps = []
    for c in range(N_CORES):
        sl = slice(c * per_b, (c + 1) * per_b)
        in_maps.append(
            {
                "input": np.ascontiguousarray(input[sl], dtype=np.float32).reshape(P, FREE),
                "target": np.ascontiguousarray(target[sl], dtype=np.float32).reshape(P, FREE),
            }
        )
    return in_maps


def kernel(input: np.ndarray, target: np.ndarray) -> np.ndarray:
    from concourse.bass_utils import run_bass_kernel_spmd

    nc = _get_program()
    in_maps = shard_inputs(input, target)
    res = run_bass_kernel_spmd(nc, in_maps, list(range(N_CORES)))
    val = res.results[0]["output"]
    return np.asarray(val, dtype=np.float32).reshape(())
